# revision 6
# baseline (speedup 1.0000x reference)
"""Fused multi-head attention block (QKV + softmax + out-proj + residual + LayerNorm)
for Trainium2, SPMD over 8 NeuronCores.

Sharding: head-parallel. Core c owns heads {2c, 2c+1} for BOTH batch elements.

Restructured vs the v1 kernel for pipelining:
  - Attention is split into 4 "tq-strip" units. Strip u covers query columns
    {512k + 128u + t : k=0..3, t<128} of each batch, i.e. the u-th 128-token
    sub-block of every destination core's 512-row block. Each strip's
    normalized output is exchanged with its own quarter-size AllToAll
    ([8,128,128] per core), so 3 of the 4 collectives (and their og loads)
    overlap attention compute instead of sitting in the tail.
  - QT is stored strip-major [128ch, 4 strip, 512] so both the Q-projection
    (strided xT read) and the S matmul rhs are clean APs.
  - Sub-unit = (strip, batch): 16 tk-steps; per step 2 S matmuls (l0/l1,
    K=64 each at partition bases 0/64 -> HW row-groups can overlap them),
    one merged exp over [128, 1024] PSUM -> bf16 SBUF, 2 AV matmuls
    (V stationary [128,65] with ones column = softmax denominator row).
  - The first sub-unit starts after only K(b0) + Q-strip0(b0) + V(b0)
    (~1/3 of QKV); the rest of QKV (all of batch 1, Q strips 1-3) is
    emitted as small "quanta" between tk-steps so it fills PE idle slots
    under the ACT-bound exp stream without head-of-line blocking.
  - LayerNorm rstd = Exp(-0.5*Ln(var+eps)): keeps every activation (Exp, Ln,
    Identity) in the single natural_log_exp table set - no ~2.7us table
    swaps anywhere.
  - PSUM: 2x S tiles [128,1024]f32 (4 banks) + O_l0/O_l1 [*,512]f32 (2) +
    2 rotating aux banks (QKV/V-JIT/Q-strip/projection) = 8 banks exactly.
"""

import sys

sys.path.insert(0, "/opt/trn_rl_repo")

import numpy as np
import ml_dtypes

BF16 = ml_dtypes.bfloat16

B, T, D = 2, 2048, 1024
H, DH = 16, 64
N_CORES = 8
LN_EPS = 1e-5
TROWS = T * B // N_CORES  # 512 output rows per core
U = 4  # tq strips

_CACHE = {}


def _build(repeat=1):
    import os
    from contextlib import ExitStack
    import concourse.bass as bass
    import concourse.tile as tile
    from concourse import bacc, mybir

    f32 = mybir.dt.float32
    bf16 = mybir.dt.bfloat16
    AF = mybir.ActivationFunctionType
    ALU = mybir.AluOpType

    def bcast(ap_src, parts):
        """Broadcast a 1-D (or row) AP across `parts` partitions (step 0)."""
        return bass.AP(tensor=ap_src.tensor, offset=ap_src.offset,
                       ap=[[0, parts]] + [list(p) for p in ap_src.ap])

    nc = bacc.Bacc("TRN2", target_bir_lowering=False, debug=False,
                   num_devices=N_CORES)

    # ---- I/O (names/shapes match _prep_inputs) ----
    xT_d = [nc.dram_tensor(f"xT{b}", [D, T], bf16, kind="ExternalInput")
            for b in range(B)]
    wqT_d = nc.dram_tensor("wqT", [D, 128], bf16, kind="ExternalInput")
    wkT_d = nc.dram_tensor("wkT", [D, 128], bf16, kind="ExternalInput")
    wvT_d = nc.dram_tensor("wvT", [D, 128], bf16, kind="ExternalInput")
    bq_d = nc.dram_tensor("bq", [128], f32, kind="ExternalInput")
    bk_d = nc.dram_tensor("bk", [128], f32, kind="ExternalInput")
    bv_d = nc.dram_tensor("bv", [128], f32, kind="ExternalInput")
    woT_d = nc.dram_tensor("woT", [D, D], bf16, kind="ExternalInput")
    bo_d = nc.dram_tensor("bo", [D], bf16, kind="ExternalInput")
    gamma_d = nc.dram_tensor("gamma", [D], f32, kind="ExternalInput")
    beta_d = nc.dram_tensor("beta", [D], f32, kind="ExternalInput")
    xres_d = nc.dram_tensor("xres", [TROWS, D], f32, kind="ExternalInput")
    out_d = nc.dram_tensor("out", [TROWS, D], f32, kind="ExternalOutput")

    NCH = 8  # 1024 / 128 contraction chunks

    def _emit_body(tc):
        ctx = ExitStack()
        persist = ctx.enter_context(tc.tile_pool(name="persist", bufs=1))
        dram = ctx.enter_context(tc.tile_pool(name="dram", bufs=1, space="DRAM"))

        # ---- prewarm the natural_log_exp table set (Ln first narrows the
        # choice to the set that also serves Exp and Identity) ----
        warm = persist.tile([1, 1], f32)
        nc.vector.memset(warm[:], 1.0)
        nc.scalar.activation(warm[:], warm[:], AF.Ln)
        nc.scalar.activation(warm[:], warm[:], AF.Exp, scale=0.125)

        # ---- input DMAs. xT b0 first (gates everything); big/late tensors on
        # the gpsimd queue after xT; weight tensors on vector. ----
        xT_sb = [persist.tile([128, NCH, T], bf16, name=f"xT{b}sb")
                 for b in range(B)]
        for ci in range(NCH):
            eng = nc.sync if ci % 2 == 0 else nc.gpsimd
            eng.dma_start(out=xT_sb[0][:, ci, :],
                          in_=xT_d[0][128 * ci:128 * (ci + 1), :])
        wqT_sb = persist.tile([128, NCH, 128], bf16)
        wkT_sb = persist.tile([128, NCH, 128], bf16)
        wvT_sb = persist.tile([128, NCH, 128], bf16)
        for w_sb, w_d in ((wkT_sb, wkT_d), (wqT_sb, wqT_d), (wvT_sb, wvT_d)):
            nc.scalar.dma_start(
                out=w_sb[:],
                in_=w_d[:].rearrange("(ci p) d -> p ci d", p=128))
        bq_sb = persist.tile([128, 1], f32)
        bk_sb = persist.tile([128, 1], f32)
        bvb_sb = persist.tile([128, 128], f32)
        nc.scalar.dma_start(out=bq_sb[:], in_=bq_d[:].rearrange("(p f) -> p f", f=1))
        nc.scalar.dma_start(out=bk_sb[:], in_=bk_d[:].rearrange("(p f) -> p f", f=1))
        nc.scalar.dma_start(out=bvb_sb[:], in_=bcast(bv_d[:], 128))
        for ci in range(NCH):
            eng = nc.sync if ci % 2 == 0 else nc.gpsimd
            eng.dma_start(out=xT_sb[1][:, ci, :],
                          in_=xT_d[1][128 * ci:128 * (ci + 1), :])

        woT_sb = persist.tile([128, NCH, D], bf16)
        for ci in range(NCH):
            nc.gpsimd.dma_start(out=woT_sb[:, ci, :],
                                in_=woT_d[128 * ci:128 * (ci + 1), :])
        bo_sb = persist.tile([1, D], bf16)
        nc.gpsimd.dma_start(out=bo_sb[:],
                            in_=bo_d[:].rearrange("(p f) -> p f", p=1))
        ones_sb = persist.tile([1, 128], bf16)
        nc.vector.memset(ones_sb[:], 1.0)
        gamma_sb = persist.tile([128, D], f32)
        beta_sb = persist.tile([128, D], f32)
        nc.gpsimd.dma_start(out=gamma_sb[:], in_=bcast(gamma_d[:], 128))
        nc.gpsimd.dma_start(out=beta_sb[:], in_=bcast(beta_d[:], 128))
        eps_sb = persist.tile([128, 1], f32)
        nc.vector.memset(eps_sb[:], LN_EPS)
        xres_sb = persist.tile([128, 4, D], f32)
        for mt in range(4):
            nc.gpsimd.dma_start(out=xres_sb[:, mt, :],
                                in_=xres_d[128 * mt:128 * (mt + 1), :])

        # QT strip-major: [128 ch, strip, 512]; KT natural [128 ch, T].
        QT_sb = [persist.tile([128, U, 512], bf16, name=f"QT{b}") for b in range(B)]
        KT_sb = [persist.tile([128, T], bf16, name=f"KT{b}") for b in range(B)]
        V_sb = [[persist.tile([128, 2, DH + 1], bf16, name=f"V{b}_{tt}")
                 for tt in range(16)] for b in range(B)]
        for b in range(B):
            for tt in range(16):
                nc.vector.memset(V_sb[b][tt][:, :, DH:DH + 1], 1.0)

        og_sb = persist.tile([128, N_CORES, U, 128], bf16)

        # a2a tiles, one pair per strip
        a2a_in = [dram.tile([N_CORES, 128, 128], bf16, name=f"a2ai{u}",
                            tag=f"a2ai{u}") for u in range(U)]
        a2a_out = [dram.tile([N_CORES, 128, 128], bf16, name=f"a2ao{u}",
                             tag=f"a2ao{u}") for u in range(U)]

        # ---- PSUM pools: 4 (S dbuf) + 2 (O) + 2 (aux) banks = 8 ----
        spool = ctx.enter_context(tc.tile_pool(name="spool", bufs=2,
                                               space="PSUM"))
        flex = ctx.enter_context(tc.tile_pool(name="flex", bufs=1,
                                              space="PSUM"))
        pp = ctx.enter_context(tc.tile_pool(name="pp", bufs=3))
        npool = ctx.enter_context(tc.tile_pool(name="npool", bufs=2))
        ln = ctx.enter_context(tc.tile_pool(name="ln", bufs=2))

        # xT strided view helper: strip u of batch b, chunk ci -> [128, 4, 128]
        def x_strip(b, ci, u):
            v = xT_sb[b].rearrange("p c (k u f) -> p c u k f", k=4, u=U)
            return v[:, ci, u, :, :]

        # ---- QKV building blocks (each appends to engine queues) ----
        def k_group_mms(b, n, ci_list, ps):
            for ci in ci_list:
                nc.tensor.matmul(
                    ps[:], wkT_sb[:, ci, :],
                    xT_sb[b][:, ci, 512 * n:512 * (n + 1)],
                    start=(ci == 0), stop=(ci == NCH - 1))

        def k_group_bias(b, n, ps):
            nc.vector.tensor_scalar(KT_sb[b][:, 512 * n:512 * (n + 1)],
                                    ps[:], bk_sb[:], None, ALU.add)

        def q_strip_mms(b, u, ci_list, ps):
            for ci in ci_list:
                nc.tensor.matmul(
                    ps[:], wqT_sb[:, ci, :], x_strip(b, ci, u),
                    start=(ci == 0), stop=(ci == NCH - 1))

        def q_strip_bias(b, u, ps):
            nc.vector.tensor_scalar(QT_sb[b][:, u, :], ps[:], bq_sb[:],
                                    None, ALU.add)

        def v_tile(b, tt):
            ps = flex.tile([128, 512], f32, tag="aux", bufs=2,
                            name=f"vps{b}_{tt}")
            for ci in range(NCH):
                nc.tensor.matmul(
                    ps[:, 0:128], xT_sb[b][:, ci, 128 * tt:128 * (tt + 1)],
                    wvT_sb[:, ci, :],
                    start=(ci == 0), stop=(ci == NCH - 1))
            nc.vector.tensor_add(
                V_sb[b][tt][:, :, 0:DH],
                ps[:, 0:128].rearrange("p (h d) -> p h d", h=2),
                bvb_sb[:].rearrange("p (h d) -> p h d", h=2))

        # ---- head: K(b0) in n-pairs (chunk-streamed), Q0(b0), V(b0) ----
        for npair in range(2):
            pss = [flex.tile([128, 512], f32, tag="aux", bufs=2,
                              name=f"kps0_{npair}_{j}") for j in range(2)]
            for ci in range(NCH):
                for j in range(2):
                    k_group_mms(0, 2 * npair + j, [ci], pss[j])
            for j in range(2):
                k_group_bias(0, 2 * npair + j, pss[j])
        qps0 = flex.tile([128, 512], f32, tag="aux", bufs=2, name="qps0")
        q_strip_mms(0, 0, range(NCH), qps0)
        q_strip_bias(0, 0, qps0)
        for tt in range(16):
            v_tile(0, tt)

        # ---- quanta: leftover QKV work interleaved into early sub-units ----
        # Each quantum is a closure; consumed one (sometimes two) per tk-step.
        _openk = {}

        def k1_quant(npair, ci):
            def fn():
                if ci == 0:
                    _openk[npair] = [
                        flex.tile([128, 512], f32, tag="aux", bufs=2,
                                  name=f"kps1_{npair}_{j}") for j in range(2)]
                pss = _openk[npair]
                for j in range(2):
                    k_group_mms(1, 2 * npair + j, [ci], pss[j])
                if ci == NCH - 1:
                    for j in range(2):
                        k_group_bias(1, 2 * npair + j, pss[j])
            return fn

        _openq = {}

        def q_quant(b, u, half):
            def fn():
                if half == 0:
                    _openq[(b, u)] = flex.tile(
                        [128, 512], f32, tag="aux", bufs=2, name=f"qps{b}_{u}")
                ps = _openq[(b, u)]
                q_strip_mms(b, u, range(4 * half, 4 * (half + 1)), ps)
                if half == 1:
                    q_strip_bias(b, u, ps)
            return fn

        # schedule[si][step] -> list of quanta
        sched = {si: {} for si in range(2 * U)}

        def put(si, step, fn):
            sched[si].setdefault(step, []).append(fn)

        for ci in range(NCH):        # si=0: K(b1) pairs
            put(0, ci, k1_quant(0, ci))
            put(0, 8 + ci, k1_quant(1, ci))
        put(0, 14, q_quant(1, 0, 0))
        put(0, 15, q_quant(1, 0, 1))
        for tt in range(16):         # si=1: V(b1) just-in-time
            put(1, tt, (lambda b, t: lambda: v_tile(b, t))(1, tt))
        put(1, 12, q_quant(0, 1, 0))
        put(1, 13, q_quant(0, 1, 1))
        qrest = [(1, 1), (0, 2), (1, 2), (0, 3), (1, 3)]
        for i, (b, u) in enumerate(qrest):  # si=2
            put(2, 2 * i, q_quant(b, u, 0))
            put(2, 2 * i + 1, q_quant(b, u, 1))

        # ---- attention sub-units ----
        for u in range(U):
            for b in range(B):
                si = 2 * u + b
                O_ps = [flex.tile([128, 512], f32, tag=f"o{l}",
                                  name=f"O{u}{b}{l}") for l in range(2)]
                prevP = None
                for tk in range(16):
                    for fn in sched[si].get(tk, ()):
                        fn()
                    S = spool.tile([128, 1024], f32, tag="s",
                                   name=f"S{u}{b}_{tk}")
                    for l in range(2):
                        lo = 64 * l
                        nc.tensor.matmul(
                            S[:, 512 * l:512 * (l + 1)],
                            KT_sb[b][lo:lo + 64, 128 * tk:128 * (tk + 1)],
                            QT_sb[b][lo:lo + 64, u, :],
                            start=True, stop=True)
                    if prevP is not None:
                        for l in range(2):
                            nc.tensor.matmul(
                                O_ps[l][0:DH + 1, :],
                                V_sb[b][tk - 1][:, l, :],
                                prevP[:, 512 * l:512 * (l + 1)],
                                start=(tk - 1 == 0), stop=False,
                                skip_group_check=True)
                    P = pp.tile([128, 1024], bf16, tag="p",
                                name=f"P{u}{b}_{tk}")
                    nc.scalar.activation(P[:], S[:], AF.Exp, scale=0.125)
                    prevP = P
                for l in range(2):
                    nc.tensor.matmul(
                        O_ps[l][0:DH + 1, :], V_sb[b][15][:, l, :],
                        prevP[:, 512 * l:512 * (l + 1)],
                        start=False, stop=True, skip_group_check=True)
                # normalize by the denominator row and stage for AllToAll #u
                for l in range(2):
                    rr = npool.tile([1, 512], f32, tag="rr")
                    nc.vector.reciprocal(rr[:], O_ps[l][DH:DH + 1, :])
                    rb = npool.tile([64, 512], f32, tag="rb")
                    nc.gpsimd.partition_broadcast(rb[:], rr[:])
                    onorm = npool.tile([64, 512], bf16, tag=f"on{l}")
                    nc.vector.tensor_mul(onorm[:], O_ps[l][0:DH, :], rb[:])
                    seng = nc.sync if l == 0 else nc.gpsimd
                    seng.dma_start(
                        out=a2a_in[u][4 * b:4 * (b + 1),
                                      64 * l:64 * (l + 1), :]
                        .rearrange("k p f -> p k f"),
                        in_=onorm[:].rearrange("p (k f) -> p k f", k=4))
            nc.gpsimd.collective_compute(
                "AllToAll", mybir.AluOpType.bypass,
                replica_groups=[list(range(N_CORES))],
                ins=[a2a_in[u][:].opt()], outs=[a2a_out[u][:].opt()])
            nc.gpsimd.dma_start(
                out=og_sb[:, 0:4, u, :],
                in_=a2a_out[u][0:4].rearrange("g p f -> p g f"))
            nc.gpsimd.dma_start(
                out=og_sb[:, 4:8, u, :],
                in_=a2a_out[u][4:8].rearrange("g p f -> p g f"))

        # ---- projection + residual + LayerNorm (mt == strip u) ----
        for mt in range(4):
            y_sb = ln.tile([128, D], f32, tag="y")
            for oc in range(2):
                ps = flex.tile([128, 512], f32, tag="aux", bufs=2,
                               name=f"pj{mt}{oc}")
                for g in range(N_CORES):
                    nc.tensor.matmul(
                        ps[:], og_sb[:, g, mt, :],
                        woT_sb[:, g, 512 * oc:512 * (oc + 1)],
                        start=(g == 0), stop=False, skip_group_check=True)
                nc.tensor.matmul(
                    ps[:], ones_sb[:], bo_sb[:, 512 * oc:512 * (oc + 1)],
                    start=False, stop=True, skip_group_check=True)
                nc.vector.tensor_add(y_sb[:, 512 * oc:512 * (oc + 1)],
                                     ps[:], xres_sb[:, mt,
                                                    512 * oc:512 * (oc + 1)])
                if oc == 0:
                    stats = ln.tile([128, 2, nc.vector.BN_STATS_DIM], f32,
                                    tag="stats")
                nc.vector.bn_stats(out=stats[:, oc, :],
                                   in_=y_sb[:, 512 * oc:512 * (oc + 1)])
            mv = ln.tile([128, nc.vector.BN_AGGR_DIM], f32, tag="mv")
            nc.vector.bn_aggr(out=mv[:], in_=stats[:])
            # rstd = exp(-0.5 * ln(var + eps)): stays in the loaded table set
            lnv = ln.tile([128, 1], f32, tag="lnv")
            nc.scalar.activation(lnv[:], mv[:, 1:2], AF.Ln, bias=eps_sb[:])
            rstd = ln.tile([128, 1], f32, tag="rstd")
            nc.scalar.activation(rstd[:], lnv[:], AF.Exp, scale=-0.5)
            negmr = ln.tile([128, 1], f32, tag="negmr")
            nc.vector.tensor_scalar(negmr[:], mv[:, 0:1], rstd[:], -1.0,
                                    ALU.mult, ALU.mult)
            yn = ln.tile([128, D], f32, tag="yn")
            nc.scalar.activation(yn[:], y_sb[:], AF.Identity,
                                 bias=negmr[:], scale=rstd[:])
            fin = ln.tile([128, D], f32, tag="fin")
            nc.vector.scalar_tensor_tensor(fin[:], yn[:], 1.0, gamma_sb[:],
                                           ALU.mult, ALU.mult)
            nc.gpsimd.tensor_add(fin[:], fin[:], beta_sb[:])
            eng = nc.sync if mt % 2 == 0 else nc.scalar
            eng.dma_start(out=out_d[128 * mt:128 * (mt + 1), :], in_=fin[:])

        ctx.close()

    trace_sim = bool(os.environ.get("KERNEL_TRACE_SIM"))
    with tile.TileContext(nc, trace_sim=trace_sim) as tc:
        for rep in range(repeat):
            if rep:
                tc.strict_bb_all_engine_barrier()
            _emit_body(tc)

    nc.compile()
    return nc


def _prep_inputs(x, Wq, bq, Wk, bk, Wv, bv, Wo, bo, gamma, beta):
    x = np.asarray(x, dtype=np.float32)
    Wq, Wk, Wv, Wo = (np.asarray(a, dtype=np.float32) for a in (Wq, Wk, Wv, Wo))
    bq, bk, bv, bo = (np.asarray(a, dtype=np.float32) for a in (bq, bk, bv, bo))
    gamma = np.asarray(gamma, dtype=np.float32)
    beta = np.asarray(beta, dtype=np.float32)

    xT = [np.ascontiguousarray(x[b].T).astype(BF16) for b in range(B)]
    # gathered channel order (same for every core): src-core-major
    ch_gath = np.empty(D, dtype=np.int64)
    for g in range(D):
        i, jj, dd = g // 128, (g % 128) // 64, g % 64
        ch_gath[g] = 16 * dd + (2 * i + jj)
    woT = np.ascontiguousarray(Wo[:, ch_gath].T).astype(BF16)
    bo_bf = bo.astype(BF16)

    in_maps = []
    for c in range(N_CORES):
        ch_loc = np.empty(128, dtype=np.int64)
        for g in range(128):
            jj, dd = g // 64, g % 64
            ch_loc[g] = 16 * dd + (2 * c + jj)
        bi, rb = c // 4, c % 4
        m = {
            "xT0": xT[0], "xT1": xT[1],
            "wqT": np.ascontiguousarray(Wq[ch_loc, :].T).astype(BF16),
            "wkT": np.ascontiguousarray(Wk[ch_loc, :].T).astype(BF16),
            "wvT": np.ascontiguousarray(Wv[ch_loc, :].T).astype(BF16),
            "bq": np.ascontiguousarray(bq[ch_loc]),
            "bk": np.ascontiguousarray(bk[ch_loc]),
            "bv": np.ascontiguousarray(bv[ch_loc]),
            "woT": woT, "bo": bo_bf, "gamma": gamma, "beta": beta,
            "xres": np.ascontiguousarray(x[bi, 512 * rb:512 * (rb + 1), :]),
        }
        in_maps.append(m)
    return in_maps


def _run(in_maps):
    from concourse.bass_utils import run_bass_kernel_spmd
    if "nc" not in _CACHE:
        _CACHE["nc"] = _build()
    res = run_bass_kernel_spmd(_CACHE["nc"], in_maps,
                               core_ids=list(range(N_CORES)))
    return res


def kernel(x, Wq, bq, Wk, bk, Wv, bv, Wo, bo, gamma, beta):
    in_maps = _prep_inputs(x, Wq, bq, Wk, bk, Wv, bv, Wo, bo, gamma, beta)
    res = _run(in_maps)
    out = np.empty((B, T, D), dtype=np.float32)
    for c in range(N_CORES):
        bi, rb = c // 4, c % 4
        out[bi, 512 * rb:512 * (rb + 1), :] = res.results[c]["out"]
    return out
